# revision 3
# baseline (speedup 1.0000x reference)
"""Bass/Trainium2 kernel for nn_BranchingGNN (bipartite GNN message passing).

Strategy (8 NeuronCores, SPMD single NEFF, per-core data differs):
  - Nodes are range-sharded: core i owns var rows [i*25000,(i+1)*25000) and
    con rows [i*12500,(i+1)*12500), padded to VR=25088 / CR=12544 rows per
    core (multiples of 128).
  - Key algebraic reformulation: messages are linear, so
        agg[d] = (sum_{e->d} h[src(e)]) @ W.T + deg(d)*b
    i.e. sum raw h rows per destination FIRST (gather + segmented sum),
    then apply the 64x64 weight in node space (12x fewer flops, and the
    gather moves raw h rows only).
  - Per direction: each core processes exactly the edges whose DESTINATION
    falls in its range. Edge sources are gathered from a replicated table
    (indirect DMA, skip-out-of-bounds padding slots), summed per dest with
    one strided DVE reduce per tile, transformed by W on the PE, combined
    with h_old + deg*b, tanh'd, and written to the core's output chunk.
  - Chunks are AllGather'd into the next direction's replicated table.
  - Per-dest slot padding: L0 capacity CAP slots per dest; dests with
    deg > CAP overflow into an L1 pre-pass whose partial sums are staged in
    extra rows appended to the gather source table, referenced by a pointer
    slot.
"""

import os
import sys
import numpy as np
from contextlib import ExitStack
from dataclasses import dataclass

sys.path.insert(0, "/opt/trn_rl_repo")

# ---------------------------------------------------------------- config

PAD_IDX = 1 << 22  # > any real table row; *64 and *256B stay in int32/uint32


@dataclass(frozen=True)
class Cfg:
    n_cores: int = 8
    nv: int = 200000          # total var nodes
    ncn: int = 100000         # total con nodes
    vf: int = 7
    cf: int = 5
    h: int = 64
    rounds: int = 2
    vr: int = 25088           # per-core var rows (mult of 512, >= nv/8)
    cr: int = 12800           # per-core con rows (mult of 512, >= ncn/8)
    cap_c: int = 15           # L0 slots per con dest (v2c direction)
    cap_v: int = 9            # L0 slots per var dest (c2v direction)
    l1_rows_c: int = 2048     # L1 rows (v2c), mult of 128
    l1_rows_v: int = 2176     # L1 rows (c2v), mult of 128
    l1_cap_c: int = 15
    l1_cap_v: int = 12
    grp_c: int = 2            # dest-tiles per gather group (v2c)
    grp_v: int = 4            # dest-tiles per gather group (c2v)

    @property
    def nvp(self):  # rows in the AllGather'd var table
        return self.n_cores * self.vr

    @property
    def ncp(self):
        return self.n_cores * self.cr

    @property
    def v_own(self):  # real rows owned per core
        return self.nv // self.n_cores

    @property
    def c_own(self):
        return self.ncn // self.n_cores


FULL = Cfg()

# ---------------------------------------------------------------- host prep


def _remap(ids, own, rows):
    """global node id -> padded table row id"""
    return (ids // own) * rows + (ids % own)


def _build_slots(src_rows, dst_local, n_dst, cap, l1_rows, l1_cap, stg_base):
    """Build L0 [n_dst, cap] and L1 [l1_rows, l1_cap] int32 index arrays for
    one core's one direction.  src_rows: table row of each edge's source;
    dst_local: local dest row in [0, n_dst); both length = n_edges_core.
    Returns (l0, l1, deg) with PAD_IDX padding; dests with deg > cap place
    their first cap-1 edges in L0, a pointer (stg_base + l1_row) in slot
    cap-1, and the rest in their L1 row."""
    order = np.argsort(dst_local, kind="stable")
    d = dst_local[order]
    s = src_rows[order]
    deg = np.bincount(d, minlength=n_dst).astype(np.int64)
    start = np.concatenate([[0], np.cumsum(deg)[:-1]])
    slot = np.arange(len(d)) - start[d]  # rank of edge within its dest

    l0 = np.full((n_dst, cap), PAD_IDX, np.int32)
    l1 = np.full((l1_rows, l1_cap), PAD_IDX, np.int32)

    big = deg > cap  # dests needing an L1 row
    n_big = int(big.sum())
    assert n_big <= l1_rows, (n_big, l1_rows)
    assert deg.max(initial=0) <= (cap - 1) + l1_cap, deg.max()
    l1_of = np.full(n_dst, -1, np.int64)
    l1_of[big] = np.arange(n_big)

    is_big_e = big[d]
    # small dests: all edges in L0. big dests: slots 0..cap-2 in L0.
    in_l0 = (~is_big_e & (slot < cap)) | (is_big_e & (slot < cap - 1))
    l0[d[in_l0], slot[in_l0]] = s[in_l0]
    # pointer slots
    l0[np.where(big)[0], cap - 1] = stg_base + l1_of[big]
    # overflow edges
    ov = is_big_e & (slot >= cap - 1)
    l1[l1_of[d[ov]], slot[ov] - (cap - 1)] = s[ov]
    return l0, l1, deg.astype(np.float32)


def prep_inputs(inputs, cfg: Cfg):
    """Full numpy preprocessing -> list of per-core input dicts."""
    c = cfg
    ev = np.asarray(inputs["edge_var"])
    ec = np.asarray(inputs["edge_con"])
    xv = np.asarray(inputs["var_features"], np.float32)
    xc = np.asarray(inputs["con_features"], np.float32)

    # padded, transposed feature arrays
    xv_t = np.zeros((c.vf, c.nvp), np.float32)
    xc_t = np.zeros((c.cf, c.ncp), np.float32)
    vrow = _remap(np.arange(c.nv), c.v_own, c.vr)
    crow = _remap(np.arange(c.ncn), c.c_own, c.cr)
    xv_t[:, vrow] = xv.T
    xc_t[:, crow] = xc.T

    ev_row = _remap(ev, c.v_own, c.vr).astype(np.int64)
    ec_row = _remap(ec, c.c_own, c.cr).astype(np.int64)

    per_core = []
    for i in range(c.n_cores):
        m_c = (ec // c.c_own) == i  # edges whose con-dest is on core i
        m_v = (ev // c.v_own) == i
        l0c, l1c, degc = _build_slots(
            ev_row[m_c], (ec[m_c] % c.c_own).astype(np.int64), c.cr,
            c.cap_c, c.l1_rows_c, c.l1_cap_c, c.nvp)
        l0v, l1v, degv = _build_slots(
            ec_row[m_v], (ev[m_v] % c.v_own).astype(np.int64), c.vr,
            c.cap_v, c.l1_rows_v, c.l1_cap_v, c.ncp)
        per_core.append(dict(
            xv_t=np.ascontiguousarray(xv_t[:, i * c.vr:(i + 1) * c.vr]),
            xc_t=np.ascontiguousarray(xc_t[:, i * c.cr:(i + 1) * c.cr]),
            idx_v2c_l0=l0c, idx_v2c_l1=l1c, deg_con=degc,
            idx_c2v_l0=l0v, idx_c2v_l1=l1v, deg_var=degv,
        ))

    # weights (shared across cores)
    w = {}
    w["w1v_t"] = np.ascontiguousarray(np.asarray(inputs["W_ve1"], np.float32).T)  # [vf,64]
    w["w2v_t"] = np.ascontiguousarray(np.asarray(inputs["W_ve2"], np.float32).T)  # [64,64]
    w["b1v"] = np.asarray(inputs["b_ve1"], np.float32).reshape(c.h, 1)
    w["b2v"] = np.asarray(inputs["b_ve2"], np.float32).reshape(c.h, 1)
    w["w1c_t"] = np.ascontiguousarray(np.asarray(inputs["W_ce1"], np.float32).T)
    w["w2c_t"] = np.ascontiguousarray(np.asarray(inputs["W_ce2"], np.float32).T)
    w["b1c"] = np.asarray(inputs["b_ce1"], np.float32).reshape(c.h, 1)
    w["b2c"] = np.asarray(inputs["b_ce2"], np.float32).reshape(c.h, 1)
    for r in range(c.rounds):
        w[f"wt_v2c_{r}"] = np.ascontiguousarray(
            np.asarray(inputs["W_v2c"], np.float32)[r].T)  # [64,64] = W.T
        w[f"wt_c2v_{r}"] = np.ascontiguousarray(
            np.asarray(inputs["W_c2v"], np.float32)[r].T)
        w[f"b_v2c_{r}"] = np.broadcast_to(
            np.asarray(inputs["b_v2c"], np.float32)[r], (128, c.h)).copy()
        w[f"b_c2v_{r}"] = np.broadcast_to(
            np.asarray(inputs["b_c2v"], np.float32)[r], (128, c.h)).copy()
    w["wro_rep"] = np.ascontiguousarray(np.tile(
        np.asarray(inputs["W_ro"], np.float32).reshape(1, c.h), (128, c.grp_v)))
    w["b_ro"] = float(np.asarray(inputs["b_ro"]).reshape(())[()]) \
        if np.asarray(inputs["b_ro"]).size == 1 else float(inputs["b_ro"][0])

    for pc in per_core:
        pc.update({k: v for k, v in w.items() if not isinstance(v, float)})
    return per_core, w["b_ro"]


# ---------------------------------------------------------------- builder

def build_nc(cfg: Cfg, b_ro: float):
    from concourse import bass, mybir, tile
    import concourse.bacc as bacc
    from concourse.masks import make_identity

    c = cfg
    f32 = mybir.dt.float32
    i32 = mybir.dt.int32
    H = c.h

    nc = bacc.Bacc("TRN2", target_bir_lowering=False, debug=False,
                   num_devices=c.n_cores)

    # ---- I/O tensors
    def inp(name, shape, dt=f32):
        return nc.dram_tensor(name, list(shape), dt, kind="ExternalInput").ap()

    xv_t = inp("xv_t", [c.vf, c.vr])
    xc_t = inp("xc_t", [c.cf, c.cr])
    idx_v2c_l0 = inp("idx_v2c_l0", [c.cr, c.cap_c], i32)
    idx_v2c_l1 = inp("idx_v2c_l1", [c.l1_rows_c, c.l1_cap_c], i32)
    deg_con = inp("deg_con", [c.cr])
    idx_c2v_l0 = inp("idx_c2v_l0", [c.vr, c.cap_v], i32)
    idx_c2v_l1 = inp("idx_c2v_l1", [c.l1_rows_v, c.l1_cap_v], i32)
    deg_var = inp("deg_var", [c.vr])
    w1v_t = inp("w1v_t", [c.vf, H]); w2v_t = inp("w2v_t", [H, H])
    b1v = inp("b1v", [H, 1]); b2v = inp("b2v", [H, 1])
    w1c_t = inp("w1c_t", [c.cf, H]); w2c_t = inp("w2c_t", [H, H])
    b1c = inp("b1c", [H, 1]); b2c = inp("b2c", [H, 1])
    wts = {}
    for r in range(c.rounds):
        wts[("v2c", r)] = (inp(f"wt_v2c_{r}", [H, H]), inp(f"b_v2c_{r}", [128, H]))
        wts[("c2v", r)] = (inp(f"wt_c2v_{r}", [H, H]), inp(f"b_c2v_{r}", [128, H]))
    wro_rep = inp("wro_rep", [128, c.grp_v * H])
    scores = nc.dram_tensor("scores", [c.vr], f32, kind="ExternalOutput").ap()

    groups = [list(range(c.n_cores))]

    with tile.TileContext(nc) as tc:
        with ExitStack() as ctx:
            dram = ctx.enter_context(tc.tile_pool(name="dram", bufs=1, space="DRAM"))
            cpool = ctx.enter_context(tc.tile_pool(name="consts", bufs=1))
            sb = ctx.enter_context(tc.tile_pool(name="sb", bufs=3))
            sb2 = ctx.enter_context(tc.tile_pool(name="sb2", bufs=2))
            ps = ctx.enter_context(tc.tile_pool(name="ps", bufs=2, space="PSUM"))

            # tables & chunks (DRAM)
            var_tab = [dram.tile([c.nvp + c.l1_rows_c, H], f32,
                                 name=f"var_tab{r}", tag=f"var_tab{r}")
                       for r in range(c.rounds)]
            con_tab = [dram.tile([c.ncp + c.l1_rows_v, H], f32,
                                 name=f"con_tab{r}", tag=f"con_tab{r}")
                       for r in range(c.rounds)]
            chunk_var = [dram.tile([c.vr, H], f32, name=f"chunk_var{j}",
                                   tag=f"chunk_var{j}") for j in range(2)]
            chunk_con = [dram.tile([c.cr, H], f32, name=f"chunk_con{j}",
                                   tag=f"chunk_con{j}") for j in range(3)]

            ident = cpool.tile([128, 128], f32, name="ident", tag="ident")
            make_identity(nc, ident)

            # ---------------- encoder: x_t [F, rows] -> chunk [rows, H]
            def encode(x_t, F, rows, w1, b1, w2, b2, out_chunk):
                w1_sb = sb2.tile([F, H], f32, name="w1_sb", tag="encw1")
                nc.sync.dma_start(w1_sb[:], w1[:])
                w2_sb = sb2.tile([H, H], f32, name="w2_sb", tag="encw2")
                nc.sync.dma_start(w2_sb[:], w2[:])
                b1_sb = sb2.tile([H, 1], f32, name="b1_sb", tag="encb1")
                nc.sync.dma_start(b1_sb[:], b1[:])
                b2_sb = sb2.tile([H, 1], f32, name="b2_sb", tag="encb2")
                nc.sync.dma_start(b2_sb[:], b2[:])
                for t in range(rows // 512):
                    xt = sb.tile([F, 512], f32, name="xt", tag="enc_xt")
                    nc.sync.dma_start(xt[:], x_t[:, t * 512:(t + 1) * 512])
                    p1 = ps.tile([H, 512], f32, name="p1", tag="mmA")
                    nc.tensor.matmul(p1[:], lhsT=w1_sb[:], rhs=xt[:],
                                     start=True, stop=True)
                    t1 = sb.tile([H, 512], f32, name="t1", tag="enc_t1")
                    nc.scalar.activation(t1[:], p1[:],
                                         mybir.ActivationFunctionType.Tanh,
                                         bias=b1_sb[:, :])
                    p2 = ps.tile([H, 512], f32, name="p2", tag="mmA")
                    nc.tensor.matmul(p2[:], lhsT=w2_sb[:], rhs=t1[:],
                                     start=True, stop=True)
                    h2 = sb.tile([H, 512], f32, name="h2", tag="enc_h2")
                    nc.vector.tensor_scalar_add(h2[:], p2[:], b2_sb[:, :])
                    hn = sb.tile([128, 4 * H], f32, name="hn", tag="enc_hn")
                    for q in range(4):
                        pt = ps.tile([128, H], f32, name="pt", tag="trp")
                        nc.tensor.transpose(
                            pt[:], h2[:, q * 128:(q + 1) * 128], ident[:H, :H])
                        nc.scalar.activation(
                            hn[:, q * H:(q + 1) * H], pt[:],
                            mybir.ActivationFunctionType.Copy)
                    # store 512 rows; row r=t*512+q*128+p -> hn[p, q*H:...]
                    nc.sync.dma_start(
                        out_chunk[t * 512:(t + 1) * 512, :].rearrange(
                            "(q p) f -> p q f", p=128), hn[:])

            encode(xv_t, c.vf, c.vr, w1v_t, b1v, w2v_t, b2v, chunk_var[0])
            encode(xc_t, c.cf, c.cr, w1c_t, b1c, w2c_t, b2c, chunk_con[0])

            def allgather(chunk, tab, rows_total):
                nc.gpsimd.collective_compute(
                    "AllGather", mybir.AluOpType.bypass,
                    replica_groups=groups,
                    ins=[chunk[:, :]],
                    outs=[tab[0:rows_total, :]],
                )

            allgather(chunk_var[0], var_tab[0], c.nvp)

            # ---------------- one message-passing direction
            def msg_pass(src_tab, src_rows_total, l1_idx, l1_rows, l1_cap,
                         l0_idx, n_dst, cap, grp, deg, w_t, b_rep,
                         h_old_chunk, out_chunk, readout=None):
                bound_l1 = src_rows_total - 1        # L1 reads real rows only
                bound = src_rows_total + l1_rows - 1  # L0 may read staging too
                src_real = src_tab[0:src_rows_total, :]
                src_all = src_tab[0:src_rows_total + l1_rows, :]
                wt_sb = sb2.tile([H, H], f32, name="wt_sb", tag="msg_wt")
                nc.sync.dma_start(wt_sb[:], w_t[:])
                brep_sb = sb2.tile([128, H], f32, name="brep_sb", tag="msg_brep")
                nc.sync.dma_start(brep_sb[:], b_rep[:])

                # L1 pre-pass: staged partial sums for high-degree dests
                for t in range(l1_rows // 128):
                    lbuf = sb.tile([128, l1_cap * H], f32, name="lbuf", tag="gbuf")
                    nc.vector.memset(lbuf[:], 0.0)
                    lidx = sb.tile([128, l1_cap], i32, name="lidx", tag="gidx")
                    nc.sync.dma_start(
                        lidx[:], l1_idx[t * 128:(t + 1) * 128, :])
                    nc.gpsimd.indirect_dma_start(
                        out=lbuf[:, :], out_offset=None,
                        in_=src_real,
                        in_offset=bass.IndirectOffsetOnAxis(
                            ap=lidx[:, :], axis=0),
                        bounds_check=bound_l1, oob_is_err=False)
                    part = sb.tile([128, H], f32, name="part", tag="l1part")
                    nc.vector.tensor_reduce(
                        part[:],
                        lbuf[:].rearrange("p (s f) -> p f s", f=H),
                        axis=mybir.AxisListType.X, op=mybir.AluOpType.add)
                    nc.sync.dma_start(
                        src_tab[src_rows_total + t * 128:
                                src_rows_total + (t + 1) * 128, :], part[:])

                # L0 phase, grp dest-tiles at a time
                ntiles = n_dst // 128
                assert ntiles % grp == 0
                for g0 in range(ntiles // grp):
                    t0 = g0 * grp
                    gbuf = sb.tile([128, grp * cap * H], f32, name="gbuf",
                                   tag="gbuf")
                    nc.vector.memset(gbuf[:], 0.0)
                    gidx = sb.tile([128, grp * cap], i32, name="gidx", tag="gidx")
                    # l0_idx rows (t*128+p) slot s -> gidx[p, t*cap+s]
                    nc.sync.dma_start(
                        gidx[:],
                        l0_idx[t0 * 128:(t0 + grp) * 128, :].rearrange(
                            "(t p) s -> p t s", p=128))
                    nc.gpsimd.indirect_dma_start(
                        out=gbuf[:, :], out_offset=None,
                        in_=src_all,
                        in_offset=bass.IndirectOffsetOnAxis(
                            ap=gidx[:, :], axis=0),
                        bounds_check=bound, oob_is_err=False)
                    G = sb.tile([128, grp * H], f32, name="G", tag="Gsum")
                    nc.vector.tensor_reduce(
                        G[:].rearrange("p (t f) -> p t f", f=H),
                        gbuf[:].rearrange("p (t s f) -> p t f s", s=cap, f=H),
                        axis=mybir.AxisListType.X, op=mybir.AluOpType.add)
                    hold = sb.tile([128, grp * H], f32, name="hold", tag="hold")
                    nc.sync.dma_start(
                        hold[:],
                        h_old_chunk[t0 * 128:(t0 + grp) * 128, :].rearrange(
                            "(t p) f -> p t f", p=128))
                    degc = sb.tile([128, grp], f32, name="degc", tag="degc")
                    nc.sync.dma_start(
                        degc[:],
                        deg[t0 * 128:(t0 + grp) * 128].rearrange(
                            "(t p) -> p t", p=128))
                    hnew = sb.tile([128, grp * H], f32, name="hnew", tag="hnew")
                    for k in range(grp):
                        gk = G[:, k * H:(k + 1) * H]
                        # G feat-major for the W matmul
                        ptr = ps.tile([H, 128], f32, name="ptr", tag="trp")
                        nc.tensor.transpose(ptr[:], gk, ident[:, :])
                        gfm = sb.tile([H, 128], f32, name="gfm", tag="gfm")
                        nc.scalar.activation(
                            gfm[:], ptr[:], mybir.ActivationFunctionType.Copy)
                        agg = ps.tile([128, H], f32, name="agg", tag="agg")
                        nc.tensor.matmul(agg[:], lhsT=gfm[:], rhs=wt_sb[:],
                                         start=True, stop=True)
                        hk = hnew[:, k * H:(k + 1) * H]
                        # hk = h_old + deg*b + agg, then tanh
                        nc.vector.tensor_scalar_mul(
                            hk, brep_sb[:], degc[:, k:k + 1])
                        nc.vector.tensor_add(
                            hk, hk, hold[:, k * H:(k + 1) * H])
                        nc.vector.tensor_add(hk, hk, agg[:])
                        nc.scalar.activation(
                            hk, hk, mybir.ActivationFunctionType.Tanh)
                    if out_chunk is not None:
                        nc.sync.dma_start(
                            out_chunk[t0 * 128:(t0 + grp) * 128, :].rearrange(
                                "(t p) f -> p t f", p=128), hnew[:])
                    if readout is not None:
                        wro_sb, sc_sb = readout
                        m = sb.tile([128, grp * H], f32, name="m", tag="romul")
                        nc.vector.tensor_mul(m[:], hnew[:], wro_sb[:])
                        nc.vector.tensor_reduce(
                            sc_sb[:, t0:t0 + grp],
                            m[:].rearrange("p (t f) -> p t f", f=H),
                            axis=mybir.AxisListType.X, op=mybir.AluOpType.add)

            seq = []
            for r in range(c.rounds):
                seq.append(("v2c", r))
                seq.append(("c2v", r))

            wro_sb = cpool.tile([128, c.grp_v * H], f32, name="wro_sb",
                                tag="wro_sb")
            nc.sync.dma_start(wro_sb[:], wro_rep[:])
            sc_sb = cpool.tile([128, c.vr // 128], f32, name="sc_sb",
                               tag="sc_sb")

            con_state = chunk_con[0]
            var_state = chunk_var[0]
            for (d, r) in seq:
                last = (d, r) == seq[-1]
                w_t, b_rep = wts[(d, r)]
                if d == "v2c":
                    out = chunk_con[r + 1]
                    msg_pass(var_tab[r], c.nvp, idx_v2c_l1, c.l1_rows_c,
                             c.l1_cap_c, idx_v2c_l0, c.cr, c.cap_c, c.grp_c,
                             deg_con, w_t, b_rep, con_state, out)
                    allgather(out, con_tab[r], c.ncp)
                    con_state = out
                else:
                    out = None if last else chunk_var[r + 1]
                    msg_pass(con_tab[r], c.ncp, idx_c2v_l1, c.l1_rows_v,
                             c.l1_cap_v, idx_c2v_l0, c.vr, c.cap_v, c.grp_v,
                             deg_var, w_t, b_rep, var_state, out,
                             readout=(wro_sb, sc_sb) if last else None)
                    if not last:
                        allgather(out, var_tab[r + 1], c.nvp)
                        var_state = out

            # readout epilogue: sc_sb [128, ntiles] -> scores [vr]
            nt = c.vr // 128
            nc.vector.tensor_scalar_add(sc_sb[:], sc_sb[:], float(b_ro))
            for half in range(2):
                w2 = nt // 2
                pt = ps.tile([w2, 128], f32, name="pt_ro", tag="trp")
                nc.tensor.transpose(
                    pt[:], sc_sb[:, half * w2:(half + 1) * w2], ident[:, :])
                so = sb.tile([w2, 128], f32, name="so", tag="so")
                nc.scalar.activation(
                    so[:], pt[:], mybir.ActivationFunctionType.Copy)
                nc.sync.dma_start(
                    scores[half * w2 * 128:(half + 1) * w2 * 128].rearrange(
                        "(q p) -> q p", p=128), so[:])

    nc.compile()
    return nc


# ---------------------------------------------------------------- runner

_CACHE = {}


def _get_nc(cfg, b_ro):
    key = (cfg, round(b_ro, 10))
    if key not in _CACHE:
        _CACHE[key] = build_nc(cfg, b_ro)
    return _CACHE[key]


def run(inputs, cfg: Cfg = FULL, trace=False):
    from concourse import bass_utils
    per_core, b_ro = prep_inputs(inputs, cfg)
    nc = _get_nc(cfg, b_ro)
    res = bass_utils.run_bass_kernel_spmd(
        nc, per_core, core_ids=list(range(cfg.n_cores)), trace=trace)
    out = np.concatenate([r["scores"][:cfg.v_own] for r in res.results])
    return out.astype(np.float32), res


def kernel(**inputs) -> np.ndarray:
    out, _ = run(inputs, FULL)
    return out



# revision 8
# speedup vs baseline: 1.4185x; 1.4185x over previous
"""Bass/Trainium2 kernel for nn_BranchingGNN (bipartite GNN message passing).

Strategy (8 NeuronCores, SPMD single NEFF, per-core data differs):
  - Nodes range-sharded: core i owns var rows [i*25000,(i+1)*25000) and con
    rows [i*12500,(i+1)*12500), padded to VR=25088 / CR=12800 rows.
  - Messages are linear, so agg[d] = (sum_{e->d} h[src(e)]) @ W.T + deg(d)*b:
    sum raw h rows per destination FIRST, then apply the 64x64 weight in node
    space.
  - Per direction, each core processes the edges whose DESTINATION it owns.
    Edge source rows are fetched with batched SWDGE dma_gather instructions
    (int16 indices force gathering per source-chunk group: 8 var groups /
    4 con chunk-pair groups). Edges are sorted by (dst tile, group) and padded
    to 128-row blocks; the per-destination segmented sum is computed on the
    PE as GT[f,d] += block[p,f]^T @ onehot[p,d] accumulating in PSUM, where
    the one-hot masks (built on DVE from per-block dst-lane ids; pad lane 255
    kills pad rows) route each edge row to its destination lane.
  - Drain per dst tile: GT -> (bf16) -> agg = GT.T @ W.T (PE), then
    hnew = tanh(h_old + deg*b + agg) batched per tile-chunk, written to the
    core's output chunk. Chunks are AllGather'd into the next direction's
    replicated table.
"""

import sys
import numpy as np
from contextlib import ExitStack
from dataclasses import dataclass

sys.path.insert(0, "/opt/trn_rl_repo")

import ml_dtypes

# ---------------------------------------------------------------- config


@dataclass(frozen=True)
class Cfg:
    n_cores: int = 8
    nv: int = 200000          # total var nodes
    ncn: int = 100000         # total con nodes
    vf: int = 7
    cf: int = 5
    h: int = 64
    rounds: int = 2
    vr: int = 25088           # per-core var rows (mult of 128)
    cr: int = 12800           # per-core con rows (mult of 128)
    tc_c: int = 10            # dst tiles per chunk, v2c (100 tiles)
    tc_v: int = 14            # dst tiles per chunk, c2v (196 tiles)
    ng_c: int = 8             # gather groups v2c (one var chunk each)
    ng_v: int = 4             # gather groups c2v (con chunk pairs)

    @property
    def nvp(self):
        return self.n_cores * self.vr

    @property
    def ncp(self):
        return self.n_cores * self.cr

    @property
    def v_own(self):
        return self.nv // self.n_cores

    @property
    def c_own(self):
        return self.ncn // self.n_cores


FULL = Cfg()

# ---------------------------------------------------------------- host prep


def _remap(ids, own, rows):
    return (ids // own) * rows + (ids % own)


def _schedule(counts, ntiles, ngroups, tc):
    """counts: [n_cores, ntiles, ngroups] edge counts.  Returns schedule
    arrays shared by all cores."""
    nblk = -(-counts.max(axis=0) // 128)          # [T, G] ceil
    nblk[:, 0] = np.maximum(nblk[:, 0], 1)        # every tile has >=1 block
    nbt = nblk.sum(axis=1)                        # [T]
    blkbase = np.concatenate([[0], np.cumsum(nbt)[:-1]])
    gpre = np.concatenate(
        [np.zeros((ntiles, 1), np.int64), np.cumsum(nblk, axis=1)[:, :-1]],
        axis=1)                                   # [T, G]
    nchunks = ntiles // tc
    cols_cg = nblk.reshape(nchunks, tc, ngroups).sum(axis=1)  # [C, G]
    # bc[t, g]: block-col prefix of tile t within its (chunk, g) segment
    bc = np.zeros((ntiles, ngroups), np.int64)
    for c in range(nchunks):
        t0 = c * tc
        bc[t0:t0 + tc] = np.concatenate(
            [np.zeros((1, ngroups), np.int64),
             np.cumsum(nblk[t0:t0 + tc], axis=0)[:-1]], axis=0)
    # segment offsets (idx positions), c-major then g
    seg_sizes = (cols_cg * 128).reshape(-1)       # [C*G]
    seg_off = np.concatenate([[0], np.cumsum(seg_sizes)[:-1]]).reshape(
        nchunks, ngroups)
    bfc0 = np.concatenate(
        [np.zeros((nchunks, 1), np.int64), np.cumsum(cols_cg, axis=1)[:, :-1]],
        axis=1)                                   # [C, G]
    return dict(nblk=nblk, nbt=nbt, blkbase=blkbase, gpre=gpre, bc=bc,
                cols_cg=cols_cg, seg_off=seg_off, bfc0=bfc0,
                ntiles=ntiles, ngroups=ngroups, tc=tc, nchunks=nchunks,
                Ltot=int(seg_sizes.sum()), NB=int(nbt.sum()))


def _streams(lidx, g, dst_local, sched):
    """Per-core: build idx16 [128, Ltot/16] and dl [128, NB] bf16."""
    T, G, tc = sched["ntiles"], sched["ngroups"], sched["tc"]
    t = dst_local >> 7
    lane = dst_local & 127
    cell = t * G + g
    order = np.lexsort((lidx, cell))
    cell_s = cell[order]
    lidx_s = lidx[order]
    lane_s = lane[order]
    cnt = np.bincount(cell_s, minlength=T * G)
    start = np.concatenate([[0], np.cumsum(cnt)[:-1]])
    rank = np.arange(len(cell_s)) - start[cell_s]
    b = rank >> 7
    r = rank & 127
    ts = cell_s // G
    gs = cell_s % G
    c = ts // tc
    pos = (sched["seg_off"][c, gs]
           + (sched["bc"][ts, gs] + b) * 128 + r)
    idx_flat = np.zeros(sched["Ltot"], np.int16)
    idx_flat[pos] = lidx_s.astype(np.int16)
    dlcol = sched["blkbase"][ts] + sched["gpre"][ts, gs] + b
    dl = np.full((128, sched["NB"]), 255.0, ml_dtypes.bfloat16)
    dl[r, dlcol] = lane_s.astype(ml_dtypes.bfloat16)
    idx16 = np.tile(
        idx_flat.reshape(-1, 16).T.reshape(16, -1), (8, 1))
    # wrap is PER SEGMENT of each dma_gather call; since every segment length
    # is a multiple of 16 and wrapping is position-local (j -> [j%16, j//16]),
    # a global wrap with segment-aligned offsets is identical per segment.
    return np.ascontiguousarray(idx16), dl


def _deg_arr(dst_local, n_dst):
    deg = np.bincount(dst_local, minlength=n_dst).astype(np.float32)
    return np.ascontiguousarray(deg.reshape(-1, 128).T)  # [128, ntiles]


def prep_inputs(inputs, cfg: Cfg):
    c = cfg
    ev = np.asarray(inputs["edge_var"]).astype(np.int64)
    ec = np.asarray(inputs["edge_con"]).astype(np.int64)
    xv = np.asarray(inputs["var_features"], np.float32)
    xc = np.asarray(inputs["con_features"], np.float32)

    xv_t = np.zeros((c.vf, c.nvp), np.float32)
    xc_t = np.zeros((c.cf, c.ncp), np.float32)
    vrow = _remap(np.arange(c.nv), c.v_own, c.vr)
    crow = _remap(np.arange(c.ncn), c.c_own, c.cr)
    xv_t[:, vrow] = xv.T
    xc_t[:, crow] = xc.T

    # ---- per-direction edge decomposition
    # v2c: dst = con (owner core), src = var (group by var chunk)
    own_c = ec // c.c_own
    dstl_c = ec % c.c_own
    g_c = ev // c.v_own
    lidx_c = ev % c.v_own
    # c2v: dst = var, src = con (group by chunk PAIR)
    own_v = ev // c.v_own
    dstl_v = ev % c.v_own
    jc = ec // c.c_own
    g_v = jc // 2
    lidx_v = (jc % 2) * c.cr + (ec % c.c_own)

    nt_c, nt_v = c.cr // 128, c.vr // 128
    counts_c = np.zeros((c.n_cores, nt_c, c.ng_c), np.int64)
    counts_v = np.zeros((c.n_cores, nt_v, c.ng_v), np.int64)
    for i in range(c.n_cores):
        m = own_c == i
        np.add.at(counts_c[i], ((dstl_c[m] >> 7), g_c[m]), 1)
        m = own_v == i
        np.add.at(counts_v[i], ((dstl_v[m] >> 7), g_v[m]), 1)
    sch_c = _schedule(counts_c, nt_c, c.ng_c, c.tc_c)
    sch_v = _schedule(counts_v, nt_v, c.ng_v, c.tc_v)

    per_core = []
    for i in range(c.n_cores):
        m = own_c == i
        idx_c, dl_c = _streams(lidx_c[m], g_c[m], dstl_c[m], sch_c)
        degc = _deg_arr(dstl_c[m], c.cr)
        m = own_v == i
        idx_v, dl_v = _streams(lidx_v[m], g_v[m], dstl_v[m], sch_v)
        degv = _deg_arr(dstl_v[m], c.vr)
        per_core.append(dict(
            xv_t=np.ascontiguousarray(xv_t[:, i * c.vr:(i + 1) * c.vr]),
            xc_t=np.ascontiguousarray(xc_t[:, i * c.cr:(i + 1) * c.cr]),
            idx_c=idx_c, dl_c=dl_c, deg_c=degc,
            idx_v=idx_v, dl_v=dl_v, deg_v=degv,
        ))

    w = {}
    w["w1v_t"] = np.ascontiguousarray(np.asarray(inputs["W_ve1"], np.float32).T)
    w["w2v_t"] = np.ascontiguousarray(np.asarray(inputs["W_ve2"], np.float32).T)
    w["b1v"] = np.asarray(inputs["b_ve1"], np.float32).reshape(c.h, 1)
    w["b2v"] = np.asarray(inputs["b_ve2"], np.float32).reshape(c.h, 1)
    w["w1c_t"] = np.ascontiguousarray(np.asarray(inputs["W_ce1"], np.float32).T)
    w["w2c_t"] = np.ascontiguousarray(np.asarray(inputs["W_ce2"], np.float32).T)
    w["b1c"] = np.asarray(inputs["b_ce1"], np.float32).reshape(c.h, 1)
    w["b2c"] = np.asarray(inputs["b_ce2"], np.float32).reshape(c.h, 1)
    for r in range(c.rounds):
        w[f"wt_v2c_{r}"] = np.ascontiguousarray(
            np.asarray(inputs["W_v2c"], np.float32)[r].T.astype(
                ml_dtypes.bfloat16))
        w[f"wt_c2v_{r}"] = np.ascontiguousarray(
            np.asarray(inputs["W_c2v"], np.float32)[r].T.astype(
                ml_dtypes.bfloat16))
        w[f"b_v2c_{r}"] = np.broadcast_to(
            np.asarray(inputs["b_v2c"], np.float32)[r], (128, c.h)).copy()
        w[f"b_c2v_{r}"] = np.broadcast_to(
            np.asarray(inputs["b_c2v"], np.float32)[r], (128, c.h)).copy()
    w["iota"] = np.broadcast_to(
        np.arange(128, dtype=ml_dtypes.bfloat16), (128, 128)).copy()
    w["wro_rep"] = np.ascontiguousarray(np.tile(
        np.asarray(inputs["W_ro"], np.float32).reshape(1, c.h),
        (128, c.tc_v)))
    b_ro = float(np.asarray(inputs["b_ro"]).reshape(-1)[0])

    for pc in per_core:
        pc.update(w)
    meta = dict(b_ro=b_ro, sch_c=sch_c, sch_v=sch_v)
    return per_core, meta


# ---------------------------------------------------------------- builder

def build_nc(cfg: Cfg, meta):
    from concourse import bass, mybir, tile
    import concourse.bacc as bacc
    from concourse.library_config import mlp

    c = cfg
    f32 = mybir.dt.float32
    bf16 = mybir.dt.bfloat16
    i16 = mybir.dt.int16
    H = c.h
    b_ro = meta["b_ro"]
    sch_c, sch_v = meta["sch_c"], meta["sch_v"]

    nc = bacc.Bacc("TRN2", target_bir_lowering=False, debug=False,
                   num_devices=c.n_cores)

    def inp(name, shape, dt=f32):
        return nc.dram_tensor(name, list(shape), dt, kind="ExternalInput").ap()

    xv_t = inp("xv_t", [c.vf, c.vr])
    xc_t = inp("xc_t", [c.cf, c.cr])
    idx_c = inp("idx_c", [128, sch_c["Ltot"] // 16], i16)
    dl_c = inp("dl_c", [128, sch_c["NB"]], bf16)
    deg_c = inp("deg_c", [128, sch_c["ntiles"]])
    idx_v = inp("idx_v", [128, sch_v["Ltot"] // 16], i16)
    dl_v = inp("dl_v", [128, sch_v["NB"]], bf16)
    deg_v = inp("deg_v", [128, sch_v["ntiles"]])
    w1v_t = inp("w1v_t", [c.vf, H]); w2v_t = inp("w2v_t", [H, H])
    b1v = inp("b1v", [H, 1]); b2v = inp("b2v", [H, 1])
    w1c_t = inp("w1c_t", [c.cf, H]); w2c_t = inp("w2c_t", [H, H])
    b1c = inp("b1c", [H, 1]); b2c = inp("b2c", [H, 1])
    wts = {}
    for r in range(c.rounds):
        wts[("v2c", r)] = (inp(f"wt_v2c_{r}", [H, H], bf16),
                           inp(f"b_v2c_{r}", [128, H]))
        wts[("c2v", r)] = (inp(f"wt_c2v_{r}", [H, H], bf16),
                           inp(f"b_c2v_{r}", [128, H]))
    iota_in = inp("iota", [128, 128], bf16)
    wro_rep = inp("wro_rep", [128, c.tc_v * H])
    scores = nc.dram_tensor("scores", [c.vr], f32, kind="ExternalOutput").ap()

    groups = [list(range(c.n_cores))]

    with tile.TileContext(nc) as tc:
        with ExitStack() as ctx:
            dram = ctx.enter_context(tc.tile_pool(name="dram", bufs=1,
                                                  space="DRAM"))
            cpool = ctx.enter_context(tc.tile_pool(name="consts", bufs=1))
            sb = ctx.enter_context(tc.tile_pool(name="sb", bufs=3))
            big = ctx.enter_context(tc.tile_pool(name="big", bufs=2))
            ps = ctx.enter_context(tc.tile_pool(name="ps", bufs=2,
                                                space="PSUM"))

            nc.gpsimd.load_library(mlp)

            var_tab = [dram.tile([c.nvp, H], f32, name=f"var_tab{r}",
                                 tag=f"var_tab{r}") for r in range(c.rounds)]
            con_tab = [dram.tile([c.ncp, H], f32, name=f"con_tab{r}",
                                 tag=f"con_tab{r}") for r in range(c.rounds)]
            chunk_var = [dram.tile([c.vr, H], f32, name=f"chunk_var{j}",
                                   tag=f"chunk_var{j}") for j in range(2)]
            chunk_con = [dram.tile([c.cr, H], f32, name=f"chunk_con{j}",
                                   tag=f"chunk_con{j}") for j in range(3)]

            from concourse.masks import make_identity
            ident = cpool.tile([128, 128], f32, name="ident", tag="ident")
            make_identity(nc, ident)
            iota_sb = cpool.tile([128, 128], bf16, name="iota_sb", tag="iota")
            nc.sync.dma_start(iota_sb[:], iota_in[:])

            # ---------------- encoder: x_t [F, rows] -> chunk [rows, H]
            def encode(x_t, F, rows, w1, b1, w2, b2, out_chunk):
                w1_sb = sb.tile([F, H], f32, name="w1_sb", tag="encw1")
                nc.sync.dma_start(w1_sb[:], w1[:])
                w2_sb = sb.tile([H, H], f32, name="w2_sb", tag="encw2")
                nc.sync.dma_start(w2_sb[:], w2[:])
                b1_sb = sb.tile([H, 1], f32, name="b1_sb", tag="encb1")
                nc.sync.dma_start(b1_sb[:], b1[:])
                b2_sb = sb.tile([H, 1], f32, name="b2_sb", tag="encb2")
                nc.sync.dma_start(b2_sb[:], b2[:])
                for t in range(rows // 512):
                    xt = sb.tile([F, 512], f32, name="xt", tag="enc_xt")
                    nc.sync.dma_start(xt[:], x_t[:, t * 512:(t + 1) * 512])
                    p1 = ps.tile([H, 512], f32, name="p1", tag="mmA")
                    nc.tensor.matmul(p1[:], lhsT=w1_sb[:], rhs=xt[:],
                                     start=True, stop=True)
                    t1 = sb.tile([H, 512], f32, name="t1", tag="enc_t1")
                    nc.scalar.activation(t1[:], p1[:],
                                         mybir.ActivationFunctionType.Tanh,
                                         bias=b1_sb[:, :])
                    p2 = ps.tile([H, 512], f32, name="p2", tag="mmA")
                    nc.tensor.matmul(p2[:], lhsT=w2_sb[:], rhs=t1[:],
                                     start=True, stop=True)
                    h2 = sb.tile([H, 512], f32, name="h2", tag="enc_h2")
                    nc.vector.tensor_scalar_add(h2[:], p2[:], b2_sb[:, :])
                    hn = sb.tile([128, 4 * H], f32, name="hn", tag="enc_hn")
                    for q in range(4):
                        pt = ps.tile([128, H], f32, name="pt", tag="trp")
                        nc.tensor.transpose(
                            pt[:], h2[:, q * 128:(q + 1) * 128], ident[:H, :H])
                        nc.scalar.activation(
                            hn[:, q * H:(q + 1) * H], pt[:],
                            mybir.ActivationFunctionType.Copy)
                    nc.sync.dma_start(
                        out_chunk[t * 512:(t + 1) * 512, :].rearrange(
                            "(q p) f -> p q f", p=128), hn[:])

            encode(xv_t, c.vf, c.vr, w1v_t, b1v, w2v_t, b2v, chunk_var[0])
            encode(xc_t, c.cf, c.cr, w1c_t, b1c, w2c_t, b2c, chunk_con[0])

            def allgather(chunk, tab, rows_total):
                nc.gpsimd.collective_compute(
                    "AllGather", mybir.AluOpType.bypass,
                    replica_groups=groups,
                    ins=[chunk[:, :]],
                    outs=[tab[0:rows_total, :]],
                )

            allgather(chunk_var[0], var_tab[0], c.nvp)

            # ---------------- one message-passing direction
            def msg_pass(sch, src_tab, rows_per_grp, idx_in, dl_in, deg_in,
                         w_t, b_rep, h_old_chunk, out_chunk, readout=None):
                T, G, TC = sch["ntiles"], sch["ngroups"], sch["tc"]
                nblk, bfc0 = sch["nblk"], sch["bfc0"]
                cols_cg, seg_off = sch["cols_cg"], sch["seg_off"]
                max_seg_cols = int((cols_cg * 128).max()) // 16
                max_stage = int(cols_cg.max())
                max_colsc = int(cols_cg.sum(axis=1).max())
                max_nbt = int(sch["nbt"].max())
                nchunks = sch["nchunks"]

                wtb_sb = sb.tile([H, H], bf16, name="wtb_sb", tag="msg_wt")
                nc.sync.dma_start(wtb_sb[:], w_t[:])
                brep_sb = sb.tile([128, H], f32, name="brep_sb", tag="msg_b")
                nc.sync.dma_start(brep_sb[:], b_rep[:])
                dl_sb = cpool.tile([128, sch["NB"]], bf16, name="dl_sb",
                                   tag=f"dl{T}")
                nc.sync.dma_start(dl_sb[:], dl_in[:])
                deg_sb = cpool.tile([128, T], f32, name="deg_sb",
                                    tag=f"deg{T}")
                nc.sync.dma_start(deg_sb[:], deg_in[:])

                for ci in range(nchunks):
                    t0 = ci * TC
                    col0 = int(seg_off[ci, 0]) // 16
                    ncols = int((cols_cg[ci] * 128).sum()) // 16
                    idx_sb = big.tile([128, max_seg_cols * G], i16,
                                      name="idx_sb", tag="gidx")
                    nc.sync.dma_start(idx_sb[:, :ncols],
                                      idx_in[:, col0:col0 + ncols])
                    bfc = big.tile([128, max_colsc * H], bf16, name="bfc",
                                   tag="bfc")
                    MAXC = 8  # block-cols per dma_gather: single_packet caps
                    #           descs/ring at 64 -> num_idxs <= 1024
                    for g in range(G):
                        cg = int(cols_cg[ci, g])
                        so = (int(seg_off[ci, g]) - int(seg_off[ci, 0])) // 16
                        stage = big.tile([128, max_stage * H], f32,
                                         name="stage", tag="stage")
                        for k0 in range(0, cg, MAXC):
                            ck = min(MAXC, cg - k0)
                            n = ck * 128
                            nc.gpsimd.dma_gather(
                                stage[:, k0 * H:(k0 + ck) * H].rearrange(
                                    "p (j h) -> p j h", h=H),
                                src_tab[g * rows_per_grp:
                                        (g + 1) * rows_per_grp, :],
                                idx_sb[:, so + k0 * 8:so + k0 * 8 + n // 16],
                                n, n, H)
                        b0 = int(bfc0[ci, g])
                        nc.scalar.activation(
                            bfc[:, b0 * H:(b0 + cg) * H], stage[:, :cg * H],
                            mybir.ActivationFunctionType.Copy)
                    hold = big.tile([128, TC * H], f32, name="hold",
                                    tag="hold")
                    nc.sync.dma_start(
                        hold[:],
                        h_old_chunk[t0 * 128:(t0 + TC) * 128, :].rearrange(
                            "(t p) f -> p t f", p=128))
                    hn = big.tile([128, TC * H], f32, name="hnew", tag="hnew")
                    for ti in range(TC):
                        t = t0 + ti
                        nbt = int(sch["nbt"][t])
                        dl0 = int(sch["blkbase"][t])
                        oh = big.tile([128, max_nbt * 128], bf16, name="oh",
                                      tag="oh")
                        nc.vector.tensor_tensor(
                            oh[:, :nbt * 128].rearrange(
                                "p (j d) -> p j d", d=128),
                            dl_sb[:, dl0:dl0 + nbt].rearrange(
                                "p (j one) -> p j one", one=1
                            ).broadcast_to([128, nbt, 128]),
                            iota_sb[:].rearrange(
                                "p (one d) -> p one d", one=1
                            ).broadcast_to([128, nbt, 128]),
                            mybir.AluOpType.is_equal)
                        gt = ps.tile([H, 128], f32, name="gt", tag="gt")
                        j = 0
                        for g in range(G):
                            base = int(bfc0[ci, g] + sch["bc"][t, g])
                            for b in range(int(nblk[t, g])):
                                col = base + b
                                nc.tensor.matmul(
                                    gt[:],
                                    lhsT=bfc[:, col * H:(col + 1) * H],
                                    rhs=oh[:, j * 128:(j + 1) * 128],
                                    start=(j == 0), stop=(j == nbt - 1))
                                j += 1
                        gfm = sb.tile([H, 128], bf16, name="gfm", tag="gfm")
                        nc.scalar.activation(
                            gfm[:], gt[:], mybir.ActivationFunctionType.Copy)
                        agg = ps.tile([128, H], f32, name="agg", tag="agg")
                        nc.tensor.matmul(agg[:], lhsT=gfm[:], rhs=wtb_sb[:],
                                         start=True, stop=True)
                        hk = hn[:, ti * H:(ti + 1) * H]
                        nc.vector.tensor_scalar_mul(
                            hk, brep_sb[:], deg_sb[:, t:t + 1])
                        nc.vector.tensor_add(hk, hk, hold[:, ti * H:(ti + 1) * H])
                        nc.vector.tensor_add(hk, hk, agg[:])
                    nc.scalar.activation(hn[:], hn[:],
                                         mybir.ActivationFunctionType.Tanh)
                    if out_chunk is not None:
                        nc.sync.dma_start(
                            out_chunk[t0 * 128:(t0 + TC) * 128, :].rearrange(
                                "(t p) f -> p t f", p=128), hn[:])
                    if readout is not None:
                        wro_sb, sc_sb = readout
                        m = sb.tile([128, TC * H], f32, name="m", tag="romul")
                        nc.vector.tensor_mul(m[:], hn[:], wro_sb[:])
                        nc.vector.tensor_reduce(
                            sc_sb[:, t0:t0 + TC],
                            m[:].rearrange("p (t f) -> p t f", f=H),
                            axis=mybir.AxisListType.X, op=mybir.AluOpType.add)

            wro_sb = cpool.tile([128, c.tc_v * H], f32, name="wro_sb",
                                tag="wro_sb")
            nc.sync.dma_start(wro_sb[:], wro_rep[:])
            sc_sb = cpool.tile([128, c.vr // 128], f32, name="sc_sb",
                               tag="sc_sb")

            seq = []
            for r in range(c.rounds):
                seq.append(("v2c", r))
                seq.append(("c2v", r))
            con_state = chunk_con[0]
            var_state = chunk_var[0]
            for (d, r) in seq:
                last = (d, r) == seq[-1]
                w_t, b_rep = wts[(d, r)]
                if d == "v2c":
                    out = chunk_con[r + 1]
                    msg_pass(sch_c, var_tab[r], c.vr, idx_c, dl_c, deg_c,
                             w_t, b_rep, con_state, out)
                    allgather(out, con_tab[r], c.ncp)
                    con_state = out
                else:
                    out = None if last else chunk_var[r + 1]
                    msg_pass(sch_v, con_tab[r], 2 * c.cr, idx_v, dl_v, deg_v,
                             w_t, b_rep, var_state, out,
                             readout=(wro_sb, sc_sb) if last else None)
                    if not last:
                        allgather(out, var_tab[r + 1], c.nvp)
                        var_state = out

            # readout epilogue: sc_sb [128, ntiles] -> scores [vr]
            nt = c.vr // 128
            nc.vector.tensor_scalar_add(sc_sb[:], sc_sb[:], float(b_ro))
            for half in range(2):
                w2 = nt // 2
                pt = ps.tile([w2, 128], f32, name="pt_ro", tag="trp")
                nc.tensor.transpose(
                    pt[:], sc_sb[:, half * w2:(half + 1) * w2], ident[:, :])
                so = sb.tile([w2, 128], f32, name="so", tag="so")
                nc.scalar.activation(
                    so[:], pt[:], mybir.ActivationFunctionType.Copy)
                nc.sync.dma_start(
                    scores[half * w2 * 128:(half + 1) * w2 * 128].rearrange(
                        "(q p) -> q p", p=128), so[:])

    nc.compile()
    return nc


# ---------------------------------------------------------------- runner

_CACHE = {}


def _get_nc(cfg, meta):
    if cfg not in _CACHE:
        _CACHE[cfg] = build_nc(cfg, meta)
    return _CACHE[cfg]


def run(inputs, cfg: Cfg = FULL, trace=False):
    from concourse import bass_utils
    per_core, meta = prep_inputs(inputs, cfg)
    nc = _get_nc(cfg, meta)
    res = bass_utils.run_bass_kernel_spmd(
        nc, per_core, core_ids=list(range(cfg.n_cores)), trace=trace)
    out = np.concatenate([r["scores"][:cfg.v_own] for r in res.results])
    return out.astype(np.float32), res


def kernel(**inputs) -> np.ndarray:
    out, _ = run(inputs, FULL)
    return out
